# revision 25
# baseline (speedup 1.0000x reference)
"""CSWin attention Bass/Trainium2 kernel (SPMD over 8 NeuronCores).

Problem: nn_CSWinAttention. B=2, H=W=56, N=2 candidates, C=128 channels,
8 heads x d=16, vertical-stripe windows Hsp=56, Wsp=7 -> 16 windows of
L=784 tokens. Plus LePE-style depthwise-3x3 rpe on the value.

Sharding: each core owns 2 windows (core c -> batch c//4, window cols
[14*(c%4), 14*(c%4)+14)). Full attention + rpe computed on-device; host
only slices/transposes/pads inputs and concatenates outputs.

v5: bf16 datapath. Q/K arrive pre-transposed from the host in the
channel-partition layouts the PE wants (ev: even heads at 32-aligned
bases; od: odd heads rotated to 0,32,64,96), V arrives both
token-partitioned (for AV) and channel-partitioned (for the rpe conv),
so the load phase is one DMA per tensor and attention starts ~10us in.
One exp per (head, chunk) over a [112,784] 2-bank PSUM tile; softmax
normalization folded into the xbar transposed-back layout. Engine
streams are in-order with blocking waits: scalar = window-0 k loads then
pure exp; Pool = pair masks only; DVE = conv/post.
Each 4-head set runs its QK^T 4-way row-tiled, AV 4-way col-tiled.
PSUM: st 3x2 banks + av 1x2 banks = 8.
"""

import numpy as np

B, Hh, Ww, Nc, Cc = 2, 56, 56, 2, 128
HEADS, Dh, WSP = 8, 16, 7
L = Hh * WSP * Nc          # 784 tokens per window
PCH = 112                  # token chunk (partition) size; 7 chunks
QC = L // PCH              # 7
H0 = 512                   # bank-aligned split of L (512*4B = 1 psum bank)
SCALE = float(Dh) ** -0.5

# per-set head order: position j -> head; av col base 32j; QK^T row base
# per the ev/od layouts (ev: even heads at 32(h/2); od rotated: heads
# 5,7,1,3 at 0,32,64,96)
SET_HEADS = [[0, 2, 1, 3], [4, 6, 5, 7]]
QK_BASE = {0: 0, 2: 32, 4: 64, 6: 96, 5: 0, 7: 32, 1: 64, 3: 96}
OD_SRC = [80, 112, 16, 48]  # channel block starts feeding od rows 0,32,64,96

_cache = {}


def _build_program():
    import concourse.bacc as bacc
    import concourse.tile as tile
    from concourse import mybir

    f32 = mybir.dt.float32
    bf16 = mybir.dt.bfloat16
    f8 = mybir.dt.float8e4
    AT = mybir.AluOpType
    AF = mybir.ActivationFunctionType

    nc = bacc.Bacc("TRN2", target_bir_lowering=False, debug=False, num_devices=8)

    qtev_d = nc.dram_tensor("qtev", [2, Cc, 2, L], f8, kind="ExternalInput")
    qtod_d = nc.dram_tensor("qtod", [2, Cc, 2, L], f8, kind="ExternalInput")
    ktev_d = nc.dram_tensor("ktev", [2, Cc, QC, 2, PCH], f8, kind="ExternalInput")
    ktod_d = nc.dram_tensor("ktod", [2, Cc, QC, 2, PCH], f8, kind="ExternalInput")
    vt_d = nc.dram_tensor("vt", [2, Cc, 8, Cc], bf16, kind="ExternalInput")
    v_d = nc.dram_tensor("v", [Hh, 2 * WSP + 2, Nc, Cc], bf16, kind="ExternalInput")
    tapw_d = nc.dram_tensor("tapw", [Cc, 9], f32, kind="ExternalInput")
    cneg_d = nc.dram_tensor("cneg", [Cc, 1], f32, kind="ExternalInput")
    cpos_d = nc.dram_tensor("cpos", [Cc, 1], f32, kind="ExternalInput")
    mask_d = nc.dram_tensor("maskblk", [PCH, PCH], bf16, kind="ExternalInput")
    out_d = nc.dram_tensor("out", [Hh, 2 * WSP, Nc, Cc], f32, kind="ExternalOutput")

    with tile.TileContext(nc) as tc:
        with (
            tc.tile_pool(name="consts", bufs=1) as consts,
            tc.tile_pool(name="io", bufs=2) as io,
            tc.tile_pool(name="tr", bufs=2) as trp,
            tc.tile_pool(name="rpe", bufs=2) as rpep,
            tc.tile_pool(name="pt", bufs=12) as ptp,
            tc.tile_pool(name="post", bufs=2) as postp,
            tc.tile_pool(name="ps_st", bufs=3, space="PSUM") as ps_st,
            tc.tile_pool(name="ps_av", bufs=1, space="PSUM") as ps_av,
        ):
            # maskblk first: the Pool masks need it ~10us in
            maskblk = consts.tile([PCH, PCH], bf16)
            nc.sync.dma_start(out=maskblk[:], in_=mask_d[:])
            tapw = consts.tile([Cc, 9], f32)
            cneg = consts.tile([Cc, 1], f32)
            cpos = consts.tile([Cc, 1], f32)

            win = [dict() for _ in range(2)]

            def emit_loads(jj, first):
                """One DMA per pre-transposed tensor for window jj.

                first window: k on the scalar queue (ahead of the exps),
                rest on sync. second window: everything on sync."""
                keng = nc.scalar if first else nc.sync

                qt_ev = trp.tile([Cc, 2, L], f8, tag="qt_ev")
                nc.sync.dma_start(out=qt_ev[:], in_=qtev_d[jj])
                qt_od = trp.tile([Cc, 2, L], f8, tag="qt_od")
                nc.sync.dma_start(out=qt_od[:], in_=qtod_d[jj])
                kt_ev = trp.tile([Cc, QC, 2, PCH], f8, tag="kt_ev")
                keng.dma_start(out=kt_ev[:], in_=ktev_d[jj])
                kt_od = trp.tile([Cc, QC, 2, PCH], f8, tag="kt_od")
                keng.dma_start(out=kt_od[:], in_=ktod_d[jj])

                # V for AV: per-head [16 cols | ones | pad] 24-blocks,
                # token-partitioned; per-chunk DMAs (AV consumes per chunk)
                x0 = WSP * jj
                v_aug = io.tile([PCH, QC, HEADS, 24], bf16, tag="v_aug")
                nc.vector.memset(v_aug[:, :, :, Dh:Dh + 1], 1.0)
                for c in range(QC):
                    nc.sync.dma_start(
                        out=v_aug[:, c, :, 0:Dh],
                        in_=v_d[8 * c:8 * c + 8, 1 + x0:1 + x0 + WSP, :, :]
                        .rearrange("y x n (h d) -> y x (n h) d", h=HEADS),
                    )
                # V channel-partitioned with x halo for the conv
                vt8 = trp.tile([Cc, 8, Cc], bf16, tag="vt8")
                nc.sync.dma_start(out=vt8[:], in_=vt_d[jj])
                win[jj].update(v_aug=v_aug, vt8=vt8, qt_ev=qt_ev,
                               qt_od=qt_od, kt_ev=kt_ev, kt_od=kt_od)

            def emit_rpe(jj):
                """depthwise-3x3 rpe for window jj, on DVE only."""
                vt8 = win[jj]["vt8"]
                vt5 = vt8[:, :, 0:126].rearrange(
                    "c b (y x n) -> c b y x n", y=7, x=9
                )
                vs_pad = rpep.tile([Cc, 58, 9], bf16, tag="vs_pad")
                nc.vector.memset(vs_pad[:], 0.0)
                nc.vector.tensor_tensor(
                    vs_pad[:, 1:57, :].rearrange("c (yb y) x -> c yb y x", y=7),
                    vt5[:, :, :, :, 0],
                    vt5[:, :, :, :, 1],
                    AT.add,
                )
                conv_a = rpep.tile([Cc, 56, 7], f32, tag="conv_a")
                conv_b = rpep.tile([Cc, 56, 7], f32, tag="conv_b")
                acc_src = None
                for t in range(9):
                    ky, kx = t // 3, t % 3
                    shifted = vs_pad[:, ky:ky + 56, kx:kx + 7]
                    dst = conv_a if t % 2 == 0 else conv_b
                    if t == 0:
                        nc.vector.tensor_scalar(
                            dst[:], shifted, tapw[:, t:t + 1], None, AT.mult
                        )
                    else:
                        nc.vector.scalar_tensor_tensor(
                            dst[:], shifted, tapw[:, t:t + 1], acc_src[:],
                            AT.mult, AT.add,
                        )
                    acc_src = dst
                cvs = rpep.tile([Cc, 56, 7], f32, tag="cvs")
                nc.vector.scalar_tensor_tensor(
                    cvs[:], vs_pad[:, 1:57, 1:8], cneg[:], acc_src[:],
                    AT.mult, AT.add,
                )
                rpe = rpep.tile([Cc, 56, 7, 2], bf16, tag="rpe")
                for n in range(2):
                    for yb in range(8):
                        nc.vector.scalar_tensor_tensor(
                            rpe[:, 7 * yb:7 * yb + 7, :, n],
                            vt5[:, yb, :, 1:8, n],
                            cpos[:],
                            cvs[:, 7 * yb:7 * yb + 7, :],
                            AT.mult, AT.add,
                        )
                # repack into 128-padded chunks for the xbar transpose-back
                rpe2 = rpep.tile([Cc, QC, Cc], bf16, tag="rpe2")
                nc.vector.tensor_copy(
                    rpe2[:, :, 0:PCH],
                    rpe[:].rearrange("c (q y) x n -> c q (y x n)", y=8),
                )
                win[jj]["rpe2"] = rpe2

            def emit_attention(jj):
                x0 = WSP * jj
                v_aug = win[jj]["v_aug"]
                qt_ev = win[jj]["qt_ev"]; qt_od = win[jj]["qt_od"]
                kt_ev = win[jj]["kt_ev"]; kt_od = win[jj]["kt_od"]

                # rpe transposed back: [128(112 tok+pad), qc, ch]
                rpe_tb = postp.tile([Cc, QC, Cc], bf16, tag="rpe_tb")
                nc.sync.dma_start(out=rpe_tb[:], in_=win[jj]["rpe2"][:],
                                  transpose=True)

                final_sb = postp.tile([PCH, QC, Cc], f32, tag="final")

                for set_i in range(2):
                    heads = SET_HEADS[set_i]
                    av = ps_av.tile([Cc, L], f32, tag="av")  # 2 banks

                    def emit_av(qc, pt_by_head):
                        # 4 col-tiled matmuls back-to-back per half so the
                        # 32-col sub-arrays run them concurrently
                        for c0, c1 in ((0, H0), (H0, L)):
                            for j, h in enumerate(heads):
                                pt = pt_by_head[h]
                                nc.tensor.matmul(
                                    av[32 * j:32 * j + Dh + 1, c0:c1],
                                    v_aug[:, qc, h, 0:Dh + 1],
                                    pt[:, c0:c1],
                                    start=(qc == 0), stop=(qc == QC - 1),
                                    tile_position=(0, 32 * j),
                                    skip_group_check=True,
                                )

                    prev_pt = None
                    for qc in range(QC):
                        if prev_pt is not None:
                            emit_av(qc - 1, prev_pt)
                        pt_by_head = {}
                        for h in heads:
                            kt = kt_ev if h % 2 == 0 else kt_od
                            qt = qt_ev if h % 2 == 0 else qt_od
                            base = QK_BASE[h]
                            st = ps_st.tile([PCH, L], f32, tag="st")
                            nc.tensor.matmul(
                                st[:, 0:H0],
                                kt[base:base + 8, qc, :, :],
                                qt[base:base + 8, :, 0:H0],
                                start=True, stop=True,
                                perf_mode=mybir.MatmulPerfMode.DoubleRow,
                                tile_position=(base, 0),
                            )
                            nc.tensor.matmul(
                                st[:, H0:L],
                                kt[base:base + 8, qc, :, :],
                                qt[base:base + 8, :, H0:L],
                                start=True, stop=True,
                                perf_mode=mybir.MatmulPerfMode.DoubleRow,
                                tile_position=(base, 0),
                            )
                            pt = ptp.tile([PCH, L], bf16, tag="pt")
                            nc.scalar.activation(
                                pt[:], st[:], AF.Exp, scale=SCALE
                            )
                            # pair mask: zero the partner entries in the
                            # diagonal 112-block (Pool carries only these)
                            blk = pt[:, qc * PCH:(qc + 1) * PCH]
                            nc.gpsimd.tensor_tensor(
                                blk, blk, maskblk[:], AT.mult
                            )
                            pt_by_head[h] = pt
                        prev_pt = pt_by_head
                    emit_av(QC - 1, prev_pt)

                    # ----- normalize in the transposed-back layout -----
                    # av rows: slot j data at 32j..32j+16, denom at 32j+16
                    av_sb = postp.tile([Cc, QC, Cc], bf16, tag="av_sb")
                    nc.vector.tensor_copy(
                        av_sb[:, :, 0:PCH],
                        av[:].rearrange("c (q t) -> c q t", t=PCH),
                    )
                    ta = postp.tile([Cc, QC, Cc], bf16, tag="ta")
                    nc.sync.dma_start(out=ta[:], in_=av_sb[:], transpose=True)
                    # denominators -> fp32 -> reciprocal (tiny: 28 cols)
                    dcp = postp.tile([PCH, QC, 4], f32, tag="dcp")
                    nc.vector.tensor_copy(
                        dcp[:],
                        ta[0:PCH].rearrange("p q (j r) -> p q j r", r=32)[
                            :, :, :, Dh
                        ],
                    )
                    rec = postp.tile([PCH, QC, 4], f32, tag="rec")
                    nc.vector.reciprocal_approx_fast(rec[:], dcp[:])
                    # out = ta * (1/denom) + rpe, per (chunk, slot)
                    for qc in range(QC):
                        for j, h in enumerate(heads):
                            nc.vector.scalar_tensor_tensor(
                                final_sb[:, qc, Dh * h:Dh * h + Dh],
                                ta[0:PCH, qc, 32 * j:32 * j + Dh],
                                rec[:, qc, j:j + 1],
                                rpe_tb[0:PCH, qc, Dh * h:Dh * h + Dh],
                                AT.mult, AT.add,
                            )

                for c in range(QC):
                    nc.sync.dma_start(
                        out=out_d[8 * c:8 * c + 8, x0:x0 + WSP, :, :].rearrange(
                            "y x n c -> y x (n c)"
                        ),
                        in_=final_sb[:, c, :],
                    )

            # ---- emission schedule (streams are in-order per engine) ----
            emit_loads(0, first=True)
            # remaining consts ride behind window-0's critical prefix
            nc.sync.dma_start(out=tapw[:], in_=tapw_d[:])
            nc.sync.dma_start(out=cneg[:], in_=cneg_d[:])
            nc.sync.dma_start(out=cpos[:], in_=cpos_d[:])
            emit_loads(1, first=False)
            emit_rpe(0)
            emit_attention(0)
            emit_rpe(1)
            emit_attention(1)

    nc.compile()
    return nc


def _host_inputs(query, key, value, conv_w):
    """Build the 8 per-core input dicts (incl. host-side transposes)."""
    from ml_dtypes import bfloat16, float8_e4m3

    query = np.ascontiguousarray(query, dtype=np.float32)
    key = np.ascontiguousarray(key, dtype=np.float32)
    value = np.ascontiguousarray(value, dtype=np.float32)
    conv_w = np.asarray(conv_w, dtype=np.float32)

    q16 = query.astype(bfloat16)
    k16 = key.astype(bfloat16)
    v16 = value.astype(bfloat16)

    tapw = conv_w[:, 0].reshape(Cc, 9).copy()
    center = conv_w[:, 0, 1, 1].reshape(Cc, 1).copy()
    cneg = np.ascontiguousarray(-center)
    maskblk = np.ones((PCH, PCH), np.float32)
    idx = np.arange(PCH)
    maskblk[idx, idx ^ 1] = 0.0

    def chan_part(x, rhs):
        """[56, 14, 2, 128] window pair -> fp8 dual-plane ev/od layouts.

        rhs=False (k, stationary): [2, 128, 7, 2, 112] — head at row base
        QK_BASE[h], partitions base..base+8 hold d%8, plane dim d//8.
        rhs=True (q, moving): [2, 128, 2, 784]."""
        t = x.reshape(QC, 8, 2, WSP, Nc, Cc)          # yc y w x n c
        t = t.transpose(2, 5, 0, 1, 3, 4)             # w c yc y x n
        T = t.reshape(2, Cc, QC, PCH).astype(float8_e4m3)
        outs = []
        for heads4 in ([0, 2, 4, 6], [5, 7, 1, 3]):
            if rhs:
                o = np.zeros((2, Cc, 2, L), float8_e4m3)
                for j, h in enumerate(heads4):
                    for pl in range(2):
                        o[:, 32 * j:32 * j + 8, pl] = T[
                            :, Dh * h + 8 * pl:Dh * h + 8 * pl + 8
                        ].reshape(2, 8, L)
            else:
                o = np.zeros((2, Cc, QC, 2, PCH), float8_e4m3)
                for j, h in enumerate(heads4):
                    for pl in range(2):
                        o[:, 32 * j:32 * j + 8, :, pl] = T[
                            :, Dh * h + 8 * pl:Dh * h + 8 * pl + 8
                        ]
            outs.append(np.ascontiguousarray(o))
        return outs

    def chan_part_v(vs):
        """haloed [56, 16, 2, 128] -> [2, 128, 8, 128] (tok 126:128 = 0)."""
        out = np.zeros((2, Cc, 8, Cc), bfloat16)
        for w in range(2):
            hw = vs[:, 7 * w:7 * w + 9]               # [56, 9, 2, 128]
            t = hw.reshape(8, 7, 9, Nc, Cc)           # b y x n c
            t = t.transpose(4, 0, 1, 2, 3)            # c b y x n
            out[w, :, :, 0:126] = t.reshape(Cc, 8, 126)
        return out

    in_maps = []
    for c in range(8):
        b, jblk = c // 4, c % 4
        xs = 14 * jblk
        v_sl = np.zeros((Hh, 16, Nc, Cc), bfloat16)
        v_sl[:, 1:15] = v16[b, :, xs:xs + 14]
        if xs - 1 >= 0:
            v_sl[:, 0] = v16[b, :, xs - 1]
        if xs + 14 < Ww:
            v_sl[:, 15] = v16[b, :, xs + 14]

        qtev, qtod = chan_part(query[b, :, xs:xs + 14], rhs=True)
        ktev, ktod = chan_part(key[b, :, xs:xs + 14], rhs=False)
        in_maps.append({
            "qtev": qtev, "qtod": qtod,
            "ktev": ktev, "ktod": ktod,
            "vt": chan_part_v(v_sl),
            "v": v_sl,
            "tapw": tapw,
            "cneg": cneg,
            "cpos": center,
            "maskblk": maskblk.astype(bfloat16),
        })
    return in_maps


def _run(in_maps, trace=False):
    from concourse.bass_utils import run_bass_kernel_spmd

    if "nc" not in _cache:
        _cache["nc"] = _build_program()
    return run_bass_kernel_spmd(
        _cache["nc"], in_maps, core_ids=list(range(8)), trace=trace
    )


def kernel(query, key, value, conv_w):
    in_maps = _host_inputs(query, key, value, conv_w)
    res = _run(in_maps)
    out = np.zeros((B, Hh, Ww, Nc, Cc), np.float32)
    for c in range(8):
        b, jblk = c // 4, c % 4
        out[b, :, 14 * jblk:14 * jblk + 14] = res.results[c]["out"]
    return out


# revision 26
# speedup vs baseline: 1.2894x; 1.2894x over previous
"""CSWin attention Bass/Trainium2 kernel (SPMD over 8 NeuronCores).

Problem: nn_CSWinAttention. B=2, H=W=56, N=2 candidates, C=128 channels,
8 heads x d=16, vertical-stripe windows Hsp=56, Wsp=7 -> 16 windows of
L=784 tokens. Plus LePE-style depthwise-3x3 rpe on the value.

Sharding: each core owns 2 windows (core c -> batch c//4, window cols
[14*(c%4), 14*(c%4)+14)). Full attention + rpe computed on-device; host
only slices/transposes/pads inputs and concatenates outputs.

v5: bf16 datapath. Q/K arrive pre-transposed from the host in the
channel-partition layouts the PE wants (ev: even heads at 32-aligned
bases; od: odd heads rotated to 0,32,64,96), V arrives both
token-partitioned (for AV) and channel-partitioned (for the rpe conv),
so the load phase is one DMA per tensor and attention starts ~10us in.
One exp per (head, chunk) over a [112,784] 2-bank PSUM tile; softmax
normalization folded into the xbar transposed-back layout. Engine
streams are in-order with blocking waits: scalar = window-0 k loads then
pure exp; Pool = pair masks only; DVE = conv/post.
Each 4-head set runs its QK^T 4-way row-tiled, AV 4-way col-tiled.
PSUM: st 3x2 banks + av 1x2 banks = 8.
"""

import numpy as np

B, Hh, Ww, Nc, Cc = 2, 56, 56, 2, 128
HEADS, Dh, WSP = 8, 16, 7
L = Hh * WSP * Nc          # 784 tokens per window
PCH = 112                  # token chunk (partition) size; 7 chunks
QC = L // PCH              # 7
H0 = 512                   # bank-aligned split of L (512*4B = 1 psum bank)
SCALE = float(Dh) ** -0.5

# per-set head order: position j -> head; av col base 32j; QK^T row base
# per the ev/od layouts (ev: even heads at 32(h/2); od rotated: heads
# 5,7,1,3 at 0,32,64,96)
SET_HEADS = [[0, 2, 1, 3], [4, 6, 5, 7]]
QK_BASE = {0: 0, 2: 32, 4: 64, 6: 96, 5: 0, 7: 32, 1: 64, 3: 96}
OD_SRC = [80, 112, 16, 48]  # channel block starts feeding od rows 0,32,64,96

_cache = {}


def _build_program():
    import concourse.bacc as bacc
    import concourse.tile as tile
    from concourse import mybir

    f32 = mybir.dt.float32
    bf16 = mybir.dt.bfloat16
    f8 = mybir.dt.float8e4
    AT = mybir.AluOpType
    AF = mybir.ActivationFunctionType

    nc = bacc.Bacc("TRN2", target_bir_lowering=False, debug=False, num_devices=8)

    qtev_d = nc.dram_tensor("qtev", [2, Cc, QC, PCH], bf16, kind="ExternalInput")
    qtod_d = nc.dram_tensor("qtod", [2, Cc, QC, PCH], bf16, kind="ExternalInput")
    ktev_d = nc.dram_tensor("ktev", [2, Cc, QC, PCH], bf16, kind="ExternalInput")
    ktod_d = nc.dram_tensor("ktod", [2, Cc, QC, PCH], bf16, kind="ExternalInput")
    vt_d = nc.dram_tensor("vt", [2, Cc, 8, Cc], bf16, kind="ExternalInput")
    v_d = nc.dram_tensor("v", [Hh, 2 * WSP + 2, Nc, Cc], bf16, kind="ExternalInput")
    tapw_d = nc.dram_tensor("tapw", [Cc, 9], f32, kind="ExternalInput")
    cneg_d = nc.dram_tensor("cneg", [Cc, 1], f32, kind="ExternalInput")
    cpos_d = nc.dram_tensor("cpos", [Cc, 1], f32, kind="ExternalInput")
    mask_d = nc.dram_tensor("maskblk", [PCH, PCH], bf16, kind="ExternalInput")
    out_d = nc.dram_tensor("out", [Hh, 2 * WSP, Nc, Cc], f32, kind="ExternalOutput")

    with tile.TileContext(nc) as tc:
        with (
            tc.tile_pool(name="consts", bufs=1) as consts,
            tc.tile_pool(name="io", bufs=2) as io,
            tc.tile_pool(name="tr", bufs=2) as trp,
            tc.tile_pool(name="rpe", bufs=2) as rpep,
            tc.tile_pool(name="pt", bufs=12) as ptp,
            tc.tile_pool(name="post", bufs=2) as postp,
            tc.tile_pool(name="ps_st", bufs=3, space="PSUM") as ps_st,
            tc.tile_pool(name="ps_av", bufs=1, space="PSUM") as ps_av,
        ):
            # maskblk first: the Pool masks need it ~10us in
            maskblk = consts.tile([PCH, PCH], bf16)
            nc.sync.dma_start(out=maskblk[:], in_=mask_d[:])
            tapw = consts.tile([Cc, 9], f32)
            cneg = consts.tile([Cc, 1], f32)
            cpos = consts.tile([Cc, 1], f32)

            win = [dict() for _ in range(2)]

            def emit_loads(jj, first):
                """One DMA per pre-transposed tensor for window jj.

                first window: k on the scalar queue (ahead of the exps),
                rest on sync. second window: everything on sync."""
                keng = nc.scalar if first else nc.sync

                qt_ev = trp.tile([Cc, QC, PCH], bf16, tag="qt_ev")
                nc.sync.dma_start(out=qt_ev[:], in_=qtev_d[jj])
                qt_od = trp.tile([Cc, QC, PCH], bf16, tag="qt_od")
                nc.sync.dma_start(out=qt_od[:], in_=qtod_d[jj])
                kt_ev = trp.tile([Cc, QC, PCH], bf16, tag="kt_ev")
                keng.dma_start(out=kt_ev[:], in_=ktev_d[jj])
                kt_od = trp.tile([Cc, QC, PCH], bf16, tag="kt_od")
                keng.dma_start(out=kt_od[:], in_=ktod_d[jj])

                # V for AV: per-head [16 cols | ones | pad] 24-blocks,
                # token-partitioned; per-chunk DMAs (AV consumes per chunk)
                x0 = WSP * jj
                v_aug = io.tile([PCH, QC, HEADS, 24], bf16, tag="v_aug")
                nc.vector.memset(v_aug[:, :, :, Dh:Dh + 1], 1.0)
                for c in range(QC):
                    nc.sync.dma_start(
                        out=v_aug[:, c, :, 0:Dh],
                        in_=v_d[8 * c:8 * c + 8, 1 + x0:1 + x0 + WSP, :, :]
                        .rearrange("y x n (h d) -> y x (n h) d", h=HEADS),
                    )
                # V channel-partitioned with x halo for the conv
                vt8 = trp.tile([Cc, 8, Cc], bf16, tag="vt8")
                nc.sync.dma_start(out=vt8[:], in_=vt_d[jj])
                win[jj].update(v_aug=v_aug, vt8=vt8, qt_ev=qt_ev,
                               qt_od=qt_od, kt_ev=kt_ev, kt_od=kt_od)

            def emit_rpe(jj):
                """depthwise-3x3 rpe for window jj, on DVE only."""
                vt8 = win[jj]["vt8"]
                vt5 = vt8[:, :, 0:126].rearrange(
                    "c b (y x n) -> c b y x n", y=7, x=9
                )
                vs_pad = rpep.tile([Cc, 58, 9], bf16, tag="vs_pad")
                nc.vector.memset(vs_pad[:], 0.0)
                nc.vector.tensor_tensor(
                    vs_pad[:, 1:57, :].rearrange("c (yb y) x -> c yb y x", y=7),
                    vt5[:, :, :, :, 0],
                    vt5[:, :, :, :, 1],
                    AT.add,
                )
                conv_a = rpep.tile([Cc, 56, 7], f32, tag="conv_a")
                conv_b = rpep.tile([Cc, 56, 7], f32, tag="conv_b")
                acc_src = None
                for t in range(9):
                    ky, kx = t // 3, t % 3
                    shifted = vs_pad[:, ky:ky + 56, kx:kx + 7]
                    dst = conv_a if t % 2 == 0 else conv_b
                    if t == 0:
                        nc.vector.tensor_scalar(
                            dst[:], shifted, tapw[:, t:t + 1], None, AT.mult
                        )
                    else:
                        nc.vector.scalar_tensor_tensor(
                            dst[:], shifted, tapw[:, t:t + 1], acc_src[:],
                            AT.mult, AT.add,
                        )
                    acc_src = dst
                cvs = rpep.tile([Cc, 56, 7], f32, tag="cvs")
                nc.vector.scalar_tensor_tensor(
                    cvs[:], vs_pad[:, 1:57, 1:8], cneg[:], acc_src[:],
                    AT.mult, AT.add,
                )
                rpe = rpep.tile([Cc, 56, 7, 2], bf16, tag="rpe")
                for n in range(2):
                    for yb in range(8):
                        nc.vector.scalar_tensor_tensor(
                            rpe[:, 7 * yb:7 * yb + 7, :, n],
                            vt5[:, yb, :, 1:8, n],
                            cpos[:],
                            cvs[:, 7 * yb:7 * yb + 7, :],
                            AT.mult, AT.add,
                        )
                # repack into 128-padded chunks for the xbar transpose-back
                rpe2 = rpep.tile([Cc, QC, Cc], bf16, tag="rpe2")
                nc.vector.tensor_copy(
                    rpe2[:, :, 0:PCH],
                    rpe[:].rearrange("c (q y) x n -> c q (y x n)", y=8),
                )
                win[jj]["rpe2"] = rpe2

            def emit_attention(jj):
                x0 = WSP * jj
                v_aug = win[jj]["v_aug"]
                qt_ev = win[jj]["qt_ev"]; qt_od = win[jj]["qt_od"]
                kt_ev = win[jj]["kt_ev"]; kt_od = win[jj]["kt_od"]

                # rpe transposed back: [128(112 tok+pad), qc, ch]
                rpe_tb = postp.tile([Cc, QC, Cc], bf16, tag="rpe_tb")
                nc.sync.dma_start(out=rpe_tb[:], in_=win[jj]["rpe2"][:],
                                  transpose=True)

                final_sb = postp.tile([PCH, QC, Cc], f32, tag="final")

                for set_i in range(2):
                    heads = SET_HEADS[set_i]
                    av = ps_av.tile([Cc, L], f32, tag="av")  # 2 banks

                    def emit_av(qc, pt_by_head):
                        # 4 col-tiled matmuls back-to-back per half so the
                        # 32-col sub-arrays run them concurrently
                        for c0, c1 in ((0, H0), (H0, L)):
                            for j, h in enumerate(heads):
                                pt = pt_by_head[h]
                                nc.tensor.matmul(
                                    av[32 * j:32 * j + Dh + 1, c0:c1],
                                    v_aug[:, qc, h, 0:Dh + 1],
                                    pt[:, c0:c1],
                                    start=(qc == 0), stop=(qc == QC - 1),
                                    tile_position=(0, 32 * j),
                                    skip_group_check=True,
                                )

                    prev_pt = None
                    for qc in range(QC):
                        if prev_pt is not None:
                            emit_av(qc - 1, prev_pt)
                        pt_by_head = {}
                        for h in heads:
                            kt = kt_ev if h % 2 == 0 else kt_od
                            qt = qt_ev if h % 2 == 0 else qt_od
                            base = QK_BASE[h]
                            st = ps_st.tile([PCH, L], f32, tag="st")
                            qtf = qt[base:base + Dh, :, :].rearrange(
                                "k a b -> k (a b)"
                            )
                            nc.tensor.matmul(
                                st[:, 0:H0],
                                kt[base:base + Dh, qc, :],
                                qtf[:, 0:H0],
                                start=True, stop=True,
                                tile_position=(base, 0),
                            )
                            nc.tensor.matmul(
                                st[:, H0:L],
                                kt[base:base + Dh, qc, :],
                                qtf[:, H0:L],
                                start=True, stop=True,
                                tile_position=(base, 0),
                            )
                            pt = ptp.tile([PCH, L], bf16, tag="pt")
                            nc.scalar.activation(
                                pt[:], st[:], AF.Exp, scale=SCALE
                            )
                            # pair mask: zero the partner entries in the
                            # diagonal 112-block (Pool carries only these)
                            blk = pt[:, qc * PCH:(qc + 1) * PCH]
                            nc.gpsimd.tensor_tensor(
                                blk, blk, maskblk[:], AT.mult
                            )
                            pt_by_head[h] = pt
                        prev_pt = pt_by_head
                    emit_av(QC - 1, prev_pt)

                    # ----- normalize in the transposed-back layout -----
                    # av rows: slot j data at 32j..32j+16, denom at 32j+16
                    av_sb = postp.tile([Cc, QC, Cc], bf16, tag="av_sb")
                    nc.vector.tensor_copy(
                        av_sb[:, :, 0:PCH],
                        av[:].rearrange("c (q t) -> c q t", t=PCH),
                    )
                    ta = postp.tile([Cc, QC, Cc], bf16, tag="ta")
                    nc.sync.dma_start(out=ta[:], in_=av_sb[:], transpose=True)
                    # denominators -> fp32 -> reciprocal (tiny: 28 cols)
                    dcp = postp.tile([PCH, QC, 4], f32, tag="dcp")
                    nc.vector.tensor_copy(
                        dcp[:],
                        ta[0:PCH].rearrange("p q (j r) -> p q j r", r=32)[
                            :, :, :, Dh
                        ],
                    )
                    rec = postp.tile([PCH, QC, 4], f32, tag="rec")
                    nc.vector.reciprocal_approx_fast(rec[:], dcp[:])
                    # out = ta * (1/denom) + rpe, per (chunk, slot)
                    for qc in range(QC):
                        for j, h in enumerate(heads):
                            nc.vector.scalar_tensor_tensor(
                                final_sb[:, qc, Dh * h:Dh * h + Dh],
                                ta[0:PCH, qc, 32 * j:32 * j + Dh],
                                rec[:, qc, j:j + 1],
                                rpe_tb[0:PCH, qc, Dh * h:Dh * h + Dh],
                                AT.mult, AT.add,
                            )

                for c in range(QC):
                    nc.sync.dma_start(
                        out=out_d[8 * c:8 * c + 8, x0:x0 + WSP, :, :].rearrange(
                            "y x n c -> y x (n c)"
                        ),
                        in_=final_sb[:, c, :],
                    )

            # ---- emission schedule (streams are in-order per engine) ----
            emit_loads(0, first=True)
            # remaining consts ride behind window-0's critical prefix
            nc.sync.dma_start(out=tapw[:], in_=tapw_d[:])
            nc.sync.dma_start(out=cneg[:], in_=cneg_d[:])
            nc.sync.dma_start(out=cpos[:], in_=cpos_d[:])
            emit_loads(1, first=False)
            emit_rpe(0)
            emit_attention(0)
            emit_rpe(1)
            emit_attention(1)

    nc.compile()
    return nc


def _host_inputs(query, key, value, conv_w):
    """Build the 8 per-core input dicts (incl. host-side transposes)."""
    from ml_dtypes import bfloat16, float8_e4m3

    query = np.ascontiguousarray(query, dtype=np.float32)
    key = np.ascontiguousarray(key, dtype=np.float32)
    value = np.ascontiguousarray(value, dtype=np.float32)
    conv_w = np.asarray(conv_w, dtype=np.float32)

    q16 = query.astype(bfloat16)
    k16 = key.astype(bfloat16)
    v16 = value.astype(bfloat16)

    tapw = conv_w[:, 0].reshape(Cc, 9).copy()
    center = conv_w[:, 0, 1, 1].reshape(Cc, 1).copy()
    cneg = np.ascontiguousarray(-center)
    maskblk = np.ones((PCH, PCH), np.float32)
    idx = np.arange(PCH)
    maskblk[idx, idx ^ 1] = 0.0

    def chan_part(x):
        """[56, 14, 2, 128] window pair -> ev/od [2, 128, 7, 112]."""
        t = x.reshape(QC, 8, 2, WSP, Nc, Cc)          # yc y w x n c
        t = t.transpose(2, 5, 0, 1, 3, 4)             # w c yc y x n
        ev = np.ascontiguousarray(t.reshape(2, Cc, QC, PCH).astype(bfloat16))
        od = np.zeros_like(ev)
        for j, s in enumerate(OD_SRC):
            od[:, 32 * j:32 * j + Dh] = ev[:, s:s + Dh]
        return ev, od

    def chan_part_v(vs):
        """haloed [56, 16, 2, 128] -> [2, 128, 8, 128] (tok 126:128 = 0)."""
        out = np.zeros((2, Cc, 8, Cc), bfloat16)
        for w in range(2):
            hw = vs[:, 7 * w:7 * w + 9]               # [56, 9, 2, 128]
            t = hw.reshape(8, 7, 9, Nc, Cc)           # b y x n c
            t = t.transpose(4, 0, 1, 2, 3)            # c b y x n
            out[w, :, :, 0:126] = t.reshape(Cc, 8, 126)
        return out

    in_maps = []
    for c in range(8):
        b, jblk = c // 4, c % 4
        xs = 14 * jblk
        v_sl = np.zeros((Hh, 16, Nc, Cc), bfloat16)
        v_sl[:, 1:15] = v16[b, :, xs:xs + 14]
        if xs - 1 >= 0:
            v_sl[:, 0] = v16[b, :, xs - 1]
        if xs + 14 < Ww:
            v_sl[:, 15] = v16[b, :, xs + 14]

        qtev, qtod = chan_part(query[b, :, xs:xs + 14])
        ktev, ktod = chan_part(key[b, :, xs:xs + 14])
        in_maps.append({
            "qtev": qtev, "qtod": qtod,
            "ktev": ktev, "ktod": ktod,
            "vt": chan_part_v(v_sl),
            "v": v_sl,
            "tapw": tapw,
            "cneg": cneg,
            "cpos": center,
            "maskblk": maskblk.astype(bfloat16),
        })
    return in_maps


def _run(in_maps, trace=False):
    from concourse.bass_utils import run_bass_kernel_spmd

    if "nc" not in _cache:
        _cache["nc"] = _build_program()
    return run_bass_kernel_spmd(
        _cache["nc"], in_maps, core_ids=list(range(8)), trace=trace
    )


def kernel(query, key, value, conv_w):
    in_maps = _host_inputs(query, key, value, conv_w)
    res = _run(in_maps)
    out = np.zeros((B, Hh, Ww, Nc, Cc), np.float32)
    for c in range(8):
        b, jblk = c // 4, c % 4
        out[b, :, 14 * jblk:14 * jblk + 14] = res.results[c]["out"]
    return out
